# revision 13
# baseline (speedup 1.0000x reference)
"""Bidirectional SAGEConv (DirSeq sum) on 8 Trainium2 NeuronCores.

Strategy (graph/data parallel, hardcoded for N=100000, E=640000, D=128):
  - Nodes sharded by range across 8 cores (12500/core); x replicated (fp16)
    to every core's DRAM as the gather table, so no collectives are needed.
  - Host side: per (core, direction) LPT-balance local nodes into 25 groups
    of <=512 (one PSUM-bank window each), bucket edges by (group, source
    chunk of 25000 rows so gather indices fit int16), sort each cell by
    source.  Cell tile counts T[g][c] are maxed across cores so all cores
    share one program; per-core shortfall is padded with TRAILING -1 gather
    indices, which the dma_gather ucode trims for free -- the gpsimd
    descriptor generator (7.8ns/idx, the hard bottleneck) only pays for real
    edges.
  - Device, per window/direction: dma_gather pulls fp16 x rows (256B) into
    SBUF edge tiles; DVE builds pure 0/1 one-hots (is_equal against a
    broadcast dst-column AP -- avoids the slow per-partition-scalar
    tensor_scalar path); TensorE accumulates feature-major agg into a
    [128,512] PSUM window (fp16 matmuls, 1 cyc/row).  Finals per 128-node
    slice: three matmuls (agg_in@wl_in, agg_out@wl_out, x@(wr_in+wr_out)),
    1/deg mean scaling applied as a per-partition ACT scale on the
    node-major products, then DVE sums + bias and DMA out.
"""

import heapq
import os
import sys

import numpy as np

sys.path.insert(0, "/opt/trn_rl_repo")

from contextlib import ExitStack

import ml_dtypes

import concourse.bacc as bacc
import concourse.tile as tile
from concourse import bass, mybir
from concourse.bass_utils import run_bass_kernel_spmd


def _maybe_install_ntff_hook():
    """The agent image's antenv lacks axon_hooks; synthesize it so
    run_bass_kernel_spmd(trace=True) can capture NTFF profiles.  Degrades
    silently when the pieces are missing (e.g. the grading environment)."""
    try:
        import importlib.util as _u

        if _u.find_spec("antenv.axon_hooks") is not None:
            return
    except Exception:
        pass
    try:
        import types

        import antenv
        from trn_agent_boot.trn_boot import _ntff_profile_via_ctypes

        hook = _ntff_profile_via_ctypes("/opt/axon/libaxon_pjrt.so")
        mod = types.ModuleType("antenv.axon_hooks")
        mod.get_axon_ntff_profile_hook = lambda: hook
        mod.set_axon_ntff_profile_hook = lambda h: None
        sys.modules["antenv.axon_hooks"] = mod
        antenv.axon_hooks = mod
    except Exception:
        pass


_maybe_install_ntff_hook()

N_NODES = 100000
N_EDGES = 640000
D = 128
NCORES = 8
NL = N_NODES // NCORES  # 12500 local nodes per core
G = 25  # windows of <=512 nodes (one PSUM bank wide)
W = 512  # window width
NLP = G * W  # 12800 padded local node slots
NCHUNK = 4
CHUNK = 25000  # source rows per gather chunk (int16-safe)
NSLICE = G * 4  # 128-node output slices

F32 = mybir.dt.float32
F16 = mybir.dt.float16
I16 = mybir.dt.int16

LAST_EXEC_NS = None
LAST_RESULTS = None

_PROGRAM_CACHE = {}


def _lpt_group(deg_total):
    """Assign NL nodes to G groups (cap W) balancing total degree."""
    order = np.argsort(-deg_total, kind="stable")
    heap = [(0, g) for g in range(G)]
    heapq.heapify(heap)
    counts = np.zeros(G, np.int64)
    node_group = np.empty(NL, np.int32)
    node_pos = np.empty(NL, np.int32)
    for n in order:
        while True:
            load, g = heapq.heappop(heap)
            if counts[g] < W:
                break
        node_group[n] = g
        node_pos[n] = counts[g]
        counts[g] += 1
        if counts[g] < W:
            heapq.heappush(heap, (load + int(deg_total[n]), g))
    return node_group, node_pos


def _cell_counts(t_loc, s_glob, node_group):
    """Edge count per (group, chunk) cell -> [G, NCHUNK]."""
    key = node_group[t_loc].astype(np.int64) * NCHUNK + s_glob // CHUNK
    return np.bincount(key, minlength=G * NCHUNK).reshape(G, NCHUNK)


def _build_dir_arrays(t_loc, s_glob, node_group, node_pos, T):
    """Slot arrays for one (core, direction).

    T: [G, NCHUNK] tiles per cell (shared across cores).  Slots are laid out
    cell-major in (w, c) order; within a cell edges are sorted by source and
    the tail is padded with idx=-1 / dc=-1.

    Returns (idx_wrapped [128, NSLOT/16] int16, dc [128, NT] f32,
    ncnt [1, G*NCHUNK] int32 real idxs per cell for num_idxs_reg)."""
    T = np.asarray(T, np.int64)
    cell_slots = T.reshape(-1) * 128
    cell_off = np.concatenate([[0], np.cumsum(cell_slots)])
    NT = int(T.sum())
    S = NT * 128

    key = node_group[t_loc].astype(np.int64) * NCHUNK + s_glob // CHUNK
    order = np.lexsort((s_glob, key))
    key_s = key[order]
    s_s = s_glob[order]
    t_s = t_loc[order]
    cnt = np.bincount(key_s, minlength=G * NCHUNK)
    within = np.arange(len(key_s), dtype=np.int64) - np.repeat(
        np.cumsum(cnt) - cnt, cnt
    )
    slot = cell_off[key_s] + within

    gidx = np.full(S, -1, np.int16)
    dcol = np.full(S, -1.0, np.float32)
    gidx[slot] = (s_s - (key_s % NCHUNK) * CHUNK).astype(np.int16)
    dcol[slot] = node_pos[t_s].astype(np.float32)

    # the gather ucode indexes the last real (non-negative) idx; an active
    # cell with zero edges on this core would break it -- inject a harmless
    # row-0 gather (dc stays -1, so its one-hot row is all zero).
    ncnt = cnt.astype(np.int32)
    for cell in np.nonzero((T.reshape(-1) > 0) & (cnt == 0))[0]:
        gidx[cell_off[cell]] = 0
        ncnt[cell] = 1

    idx_w = np.ascontiguousarray(np.tile(gidx.reshape(S // 16, 16).T, (8, 1)))
    dc = np.ascontiguousarray(dcol.reshape(NT, 128).T)
    return idx_w, dc, ncnt.reshape(1, G * NCHUNK)


def _build_program(T_i, T_o):
    """T_i/T_o: tuple[G*NCHUNK] tiles-per-cell for each direction."""
    key = (T_i, T_o)
    if key in _PROGRAM_CACHE:
        return _PROGRAM_CACHE[key]

    T = {
        "i": np.asarray(T_i, np.int64).reshape(G, NCHUNK),
        "o": np.asarray(T_o, np.int64).reshape(G, NCHUNK),
    }
    NT = {d: int(T[d].sum()) for d in T}
    TW = {d: T[d].sum(axis=1) for d in T}  # tiles per window
    TWmax = {d: int(TW[d].max()) for d in T}
    # slot offset of each cell, in tiles
    tile_off = {
        d: np.concatenate([[0], np.cumsum(T[d].reshape(-1))]).reshape(-1) for d in T
    }

    nc = bacc.Bacc()
    xf16 = nc.declare_dram_parameter("xf16", [N_NODES, D], F16, isOutput=False)
    dram = {}
    for d in ("i", "o"):
        dram[f"idx_{d}"] = nc.declare_dram_parameter(
            f"idx_{d}", [128, NT[d] * 8], I16, isOutput=False
        )
        dram[f"dc_{d}"] = nc.declare_dram_parameter(
            f"dc_{d}", [128, NT[d]], F32, isOutput=False
        )
        dram[f"rcn_{d}"] = nc.declare_dram_parameter(
            f"rcn_{d}", [128, NSLICE], F32, isOutput=False
        )
        dram[f"ncnt_{d}"] = nc.declare_dram_parameter(
            f"ncnt_{d}", [1, G * NCHUNK], mybir.dt.int32, isOutput=False
        )
    colidx = nc.declare_dram_parameter("colidx", [128, W], F32, isOutput=False)
    xt = nc.declare_dram_parameter("xt", [D, NLP], F16, isOutput=False)
    wl_i = nc.declare_dram_parameter("wl_i", [D, D], F16, isOutput=False)
    wl_o = nc.declare_dram_parameter("wl_o", [D, D], F16, isOutput=False)
    wrs = nc.declare_dram_parameter("wrs", [D, D], F16, isOutput=False)
    bias = nc.declare_dram_parameter("bias", [128, D], F32, isOutput=False)
    y = nc.declare_dram_parameter("y", [NLP, D], F32, isOutput=True)

    AL = mybir.AluOpType
    ACTF = mybir.ActivationFunctionType
    with tile.TileContext(nc) as tc, ExitStack() as ctx:
        ep = ctx.enter_context
        const_pool = ep(tc.tile_pool(name="consts", bufs=1))
        msg_pool = {
            "i": ep(tc.tile_pool(name="msg_i", bufs=2)),
            "o": ep(tc.tile_pool(name="msg_o", bufs=2)),
        }
        idx_pool = ep(tc.tile_pool(name="idxs", bufs=4))
        s_pool = ep(tc.tile_pool(name="onehot", bufs=4))
        aggsb_pool = ep(tc.tile_pool(name="aggsb", bufs=4))
        z_pool = ep(tc.tile_pool(name="zsb", bufs=4))
        y_pool = ep(tc.tile_pool(name="ysb", bufs=6))
        agg_ps = {
            "i": ep(tc.tile_pool(name="aggps_i", bufs=2, space="PSUM")),
            "o": ep(tc.tile_pool(name="aggps_o", bufs=2, space="PSUM")),
        }
        zps_pool = ep(tc.tile_pool(name="zps", bufs=2, space="PSUM"))

        # resident constants
        dc_sb = {}
        rcn_sb = {}
        ncnt_sb = {}
        for d in ("i", "o"):
            dc_sb[d] = const_pool.tile([128, NT[d]], F32, name=f"dc_{d}")
            nc.sync.dma_start(dc_sb[d][:], dram[f"dc_{d}"][:])
            rcn_sb[d] = const_pool.tile([128, NSLICE], F32, name=f"rcn_{d}")
            nc.sync.dma_start(rcn_sb[d][:], dram[f"rcn_{d}"][:])
            ncnt_sb[d] = const_pool.tile(
                [1, G * NCHUNK], mybir.dt.int32, name=f"ncnt_{d}"
            )
            nc.sync.dma_start(ncnt_sb[d][:], dram[f"ncnt_{d}"][:])
        gcnt_reg = nc.alloc_register(mybir.EngineType.Pool, "gcnt")
        colidx_sb = const_pool.tile([128, W], F32)
        nc.sync.dma_start(colidx_sb[:], colidx[:])
        xt_sb = const_pool.tile([D, NLP], F16)
        nc.sync.dma_start(xt_sb[:], xt[:])
        wl_sb = {}
        for d, dr in (("i", wl_i), ("o", wl_o)):
            wl_sb[d] = const_pool.tile([D, D], F16, name=f"wl_{d}")
            nc.sync.dma_start(wl_sb[d][:], dr[:])
        wrs_sb = const_pool.tile([D, D], F16)
        nc.sync.dma_start(wrs_sb[:], wrs[:])
        bias_sb = const_pool.tile([128, D], F32)
        nc.sync.dma_start(bias_sb[:], bias[:])

        for w in range(G):
            aggsb = {}
            for d in ("i", "o"):
                ntw = int(TW[d][w])
                m = msg_pool[d].tile([128, TWmax[d], D], F16, name=f"msg_{d}")
                # padded (never-gathered) slots would otherwise read NaN-init
                # SBUF and poison the matmul through one-hot zeros (NaN*0).
                nc.vector.memset(m[:], 0)
                coff = 0
                for c in range(NCHUNK):
                    t_wc = int(T[d][w, c])
                    if t_wc == 0:
                        continue
                    n_idx = t_wc * 128
                    slot0 = int(tile_off[d][w * NCHUNK + c]) * 128
                    it = idx_pool.tile([128, n_idx // 16], I16, name="idxt")
                    nc.sync.dma_start(
                        it[:],
                        dram[f"idx_{d}"][:, slot0 // 16 : (slot0 + n_idx) // 16],
                    )
                    cell = w * NCHUNK + c
                    nc.gpsimd.reg_load(gcnt_reg, ncnt_sb[d][0:1, cell : cell + 1])
                    nc.gpsimd.dma_gather(
                        out_ap=m[:, coff : coff + t_wc, :],
                        in_ap=xf16[c * CHUNK : (c + 1) * CHUNK, :],
                        idxs_ap=it[:],
                        num_idxs=n_idx,
                        num_idxs_reg=gcnt_reg,
                        elem_size=D,
                        single_packet=False,
                    )
                    coff += t_wc

                ps = agg_ps[d].tile([128, W], F32, name=f"aggps_{d}")
                base_t = int(tile_off[d][w * NCHUNK])
                for t in range(ntw):
                    S = s_pool.tile([128, W], F16, name="onehot")
                    nc.vector.tensor_tensor(
                        S[:],
                        colidx_sb[:],
                        dc_sb[d][:, base_t + t : base_t + t + 1].broadcast_to(
                            [128, W]
                        ),
                        AL.is_equal,
                    )
                    nc.tensor.matmul(
                        ps[:],
                        m[:, t, :],
                        S[:],
                        start=(t == 0),
                        stop=(t == ntw - 1),
                        skip_group_check=True,
                    )
                at = aggsb_pool.tile([128, W], F16, name=f"aggsb_{d}")
                nc.scalar.activation(at[:], ps[:], ACTF.Copy)
                aggsb[d] = at

            for s in range(4):
                nsl = w * 4 + s
                ps3 = zps_pool.tile([128, 3 * D], F32)
                nc.tensor.matmul(
                    ps3[:, 0:D], aggsb["i"][:, s * 128 : (s + 1) * 128], wl_sb["i"][:],
                    start=True, stop=True, skip_group_check=True,
                )
                nc.tensor.matmul(
                    ps3[:, D : 2 * D], aggsb["o"][:, s * 128 : (s + 1) * 128],
                    wl_sb["o"][:],
                    start=True, stop=True, skip_group_check=True,
                )
                nc.tensor.matmul(
                    ps3[:, 2 * D : 3 * D], xt_sb[:, nsl * 128 : (nsl + 1) * 128],
                    wrs_sb[:],
                    start=True, stop=True, skip_group_check=True,
                )
                zA = z_pool.tile([128, D], F32, name="zA")
                nc.scalar.activation(
                    zA[:], ps3[:, 0:D], ACTF.Copy,
                    scale=rcn_sb["i"][:, nsl : nsl + 1],
                )
                zB = z_pool.tile([128, D], F32, name="zB")
                nc.scalar.activation(
                    zB[:], ps3[:, D : 2 * D], ACTF.Copy,
                    scale=rcn_sb["o"][:, nsl : nsl + 1],
                )
                y1 = y_pool.tile([128, D], F32, name="y1")
                nc.vector.tensor_tensor(y1[:], ps3[:, 2 * D : 3 * D], bias_sb[:], AL.add)
                y2 = y_pool.tile([128, D], F32, name="y2")
                nc.vector.tensor_tensor(y2[:], zA[:], zB[:], AL.add)
                y3 = y_pool.tile([128, D], F32, name="y3")
                nc.vector.tensor_tensor(y3[:], y1[:], y2[:], AL.add)
                nc.sync.dma_start(y[nsl * 128 : (nsl + 1) * 128, :], y3[:])

    nc.compile()
    _PROGRAM_CACHE[key] = nc
    return nc


def kernel(x, ei, w_l_in, b_l_in, w_r_in, w_l_out, b_l_out, w_r_out):
    global LAST_EXEC_NS, LAST_RESULTS

    x = np.ascontiguousarray(np.asarray(x, dtype=np.float32))
    ei = np.asarray(ei)
    src = ei[0].astype(np.int64)
    dst = ei[1].astype(np.int64)

    xf16_np = np.ascontiguousarray(x.astype(np.float16))
    w_l_in = np.asarray(w_l_in, np.float32)
    w_l_out = np.asarray(w_l_out, np.float32)
    wrs_np = np.ascontiguousarray(
        (np.asarray(w_r_in, np.float32) + np.asarray(w_r_out, np.float32)).T
    ).astype(np.float16)
    wl_i_np = np.ascontiguousarray(w_l_in.T).astype(np.float16)
    wl_o_np = np.ascontiguousarray(w_l_out.T).astype(np.float16)
    b_sum = np.asarray(b_l_in, np.float32) + np.asarray(b_l_out, np.float32)
    bias_np = np.ascontiguousarray(np.broadcast_to(b_sum[None, :], (128, D)))
    colidx_np = np.ascontiguousarray(
        np.broadcast_to(np.arange(W, dtype=np.float32)[None, :], (128, W))
    )

    src_core = src // NL
    dst_core = dst // NL

    per_core = []
    cellmax = {
        "i": np.zeros((G, NCHUNK), np.int64),
        "o": np.zeros((G, NCHUNK), np.int64),
    }
    for k in range(NCORES):
        base = k * NL
        m_in = dst_core == k
        t_in = (dst[m_in] - base).astype(np.int64)
        s_in = src[m_in]
        m_out = src_core == k
        t_out = (src[m_out] - base).astype(np.int64)
        s_out = dst[m_out]

        deg_i = np.bincount(t_in, minlength=NL)
        deg_o = np.bincount(t_out, minlength=NL)
        node_group, node_pos = _lpt_group(deg_i + deg_o)
        cellmax["i"] = np.maximum(cellmax["i"], _cell_counts(t_in, s_in, node_group))
        cellmax["o"] = np.maximum(cellmax["o"], _cell_counts(t_out, s_out, node_group))
        per_core.append((base, t_in, s_in, t_out, s_out, deg_i, deg_o,
                         node_group, node_pos))

    T_i = tuple(int(v) for v in (-(-cellmax["i"] // 128)).reshape(-1))
    T_o = tuple(int(v) for v in (-(-cellmax["o"] // 128)).reshape(-1))

    in_maps = []
    perms = []
    for k in range(NCORES):
        (base, t_in, s_in, t_out, s_out, deg_i, deg_o,
         node_group, node_pos) = per_core[k]
        gi_i, dc_i, ncnt_i = _build_dir_arrays(
            t_in, s_in, node_group, node_pos, np.asarray(T_i).reshape(G, NCHUNK)
        )
        gi_o, dc_o, ncnt_o = _build_dir_arrays(
            t_out, s_out, node_group, node_pos, np.asarray(T_o).reshape(G, NCHUNK)
        )

        slot_of_node = node_group.astype(np.int64) * W + node_pos
        perm = np.full(NLP, -1, np.int64)
        perm[slot_of_node] = np.arange(NL)
        perms.append(perm)

        valid = perm >= 0
        xt_np = np.zeros((D, NLP), np.float16)
        xt_np[:, valid] = x[base + perm[valid]].T.astype(np.float16)

        def rcn_of(deg):
            rc_node = 1.0 / np.maximum(deg.astype(np.float32), 1.0)
            rcn_slot = np.ones(NLP, np.float32)
            rcn_slot[valid] = rc_node[perm[valid]]
            # [128, NSLICE]: column nsl, partition p -> node slot nsl*128+p
            return np.ascontiguousarray(rcn_slot.reshape(NSLICE, 128).T)

        in_maps.append(
            {
                "xf16": xf16_np,
                "idx_i": gi_i,
                "dc_i": dc_i,
                "rcn_i": rcn_of(deg_i),
                "ncnt_i": ncnt_i,
                "idx_o": gi_o,
                "dc_o": dc_o,
                "rcn_o": rcn_of(deg_o),
                "ncnt_o": ncnt_o,
                "colidx": colidx_np,
                "xt": xt_np,
                "wl_i": wl_i_np,
                "wl_o": wl_o_np,
                "wrs": wrs_np,
                "bias": bias_np,
            }
        )

    nc = _build_program(T_i, T_o)
    trace = bool(os.environ.get("BASS_TRACE"))
    res = run_bass_kernel_spmd(nc, in_maps, list(range(NCORES)), trace=trace)
    LAST_EXEC_NS = res.exec_time_ns
    LAST_RESULTS = res

    out = np.empty((N_NODES, D), np.float32)
    for k in range(NCORES):
        yk = np.asarray(res.results[k]["y"])
        perm = perms[k]
        valid = perm >= 0
        out[k * NL + perm[valid]] = yk[valid]
    return out
